# revision 1
# baseline (speedup 1.0000x reference)
"""Trainium2 Bass kernel: dense transformer block (LN1-attn-LN2-FFN, causal, 16 heads).

Sharding (8 NeuronCores, SPMD one graph):
  - core j: token-parallel for LN/FFN/residual: owns tokens [512l, 512(l+1))
    of batch g, where g, l = divmod(j, 4)
  - attention head-parallel with cyclic head-batch assignment: core j computes
    head pair {2m, 2m+1}, m = (j + 4b) % 8, for EACH batch b over the full
    2048-token sequence. Uniform causal loop structure on every core; all
    per-core variation (which heads / which tokens) lives in the input data.
  - comm: 8-core AllGather of LN1^T output (QKV sees all tokens), 8-core
    AllToAll of normalized attention^T (head-shard -> token-shard). The
    receive-side head permutation is folded into host-permuted wo rows.
  - matmuls bf16 (f32 accumulate); residual stream f32; softmax without
    max-subtraction (scores are O(1) for this problem scale).
  - LN gains/biases, 1/sqrt(dk), and bv are folded into weights host-side.
"""

import numpy as np
import ml_dtypes

import concourse.bass as bass
import concourse.tile as tile
from concourse import bacc, mybir
from concourse.bass_utils import run_bass_kernel_spmd

F32 = mybir.dt.float32
BF16 = mybir.dt.bfloat16
AF = mybir.ActivationFunctionType

D = 1024
DFF = 4096
B = 2
S = 2048
NCORES = 8
GRP = 4
TOK = 512        # tokens per core (FFN/LN shard)
EPS = 1e-5

AG_IN = D * TOK          # bf16 elems contributed to xln AllGather
A2A_N = NCORES * 128 * TOK   # total elems in the 8-way AllToAll


def build_nc():
    nc = bacc.Bacc("TRN2", target_bir_lowering=False, debug=False,
                   num_devices=NCORES)

    x_own = nc.dram_tensor("x_own", [TOK, D], F32, kind="ExternalInput").ap()
    wq = nc.dram_tensor("wq", [D, B, 128], BF16, kind="ExternalInput").ap()
    wk = nc.dram_tensor("wk", [D, B, 128], BF16, kind="ExternalInput").ap()
    wv = nc.dram_tensor("wv", [D, B, 128], BF16, kind="ExternalInput").ap()
    bq = nc.dram_tensor("bq", [B, 128], F32, kind="ExternalInput").ap()
    bk = nc.dram_tensor("bk", [B, 128], F32, kind="ExternalInput").ap()
    wo = nc.dram_tensor("wo", [D, D], BF16, kind="ExternalInput").ap()
    bo = nc.dram_tensor("bo", [D], F32, kind="ExternalInput").ap()
    w1 = nc.dram_tensor("w1", [D, DFF], BF16, kind="ExternalInput").ap()
    b1 = nc.dram_tensor("b1", [DFF], F32, kind="ExternalInput").ap()
    w2 = nc.dram_tensor("w2", [DFF, D], BF16, kind="ExternalInput").ap()
    b2 = nc.dram_tensor("b2", [D], F32, kind="ExternalInput").ap()
    id128 = nc.dram_tensor("id128", [128, 128], BF16, kind="ExternalInput").ap()
    me = nc.dram_tensor("me", [128, 256], BF16, kind="ExternalInput").ap()
    mo = nc.dram_tensor("mo", [128, 256], BF16, kind="ExternalInput").ap()
    out = nc.dram_tensor("out", [TOK, D], F32, kind="ExternalOutput").ap()

    rg = [list(range(NCORES))]

    with tile.TileContext(nc) as tc:
        with (
            tc.tile_pool(name="persist", bufs=1) as pp,
            tc.tile_pool(name="stage", bufs=2) as stg,
            tc.tile_pool(name="stats", bufs=4) as stp,
            tc.tile_pool(name="ptp", bufs=6) as ptp,
            tc.tile_pool(name="nbp", bufs=2) as nbp,
            tc.tile_pool(name="rp", bufs=2) as rp,
            tc.tile_pool(name="psA", bufs=2, space="PSUM") as psA,
            tc.tile_pool(name="psT", bufs=1, space="PSUM") as psT,
            tc.tile_pool(name="psS", bufs=2, space="PSUM") as psS,
            tc.tile_pool(name="psO", bufs=1, space="PSUM") as psO,
            tc.tile_pool(name="dram", bufs=1, space="DRAM") as dp,
        ):
            # ---- constants ----
            id_sb = pp.tile([128, 128], BF16, name="id_sb")
            nc.sync.dma_start(out=id_sb, in_=id128)
            me_sb = pp.tile([128, 256], BF16, name="me_sb")
            nc.sync.dma_start(out=me_sb, in_=me)
            mo_sb = pp.tile([128, 256], BF16, name="mo_sb")
            nc.sync.dma_start(out=mo_sb, in_=mo)
            ones_sb = pp.tile([1, 128], F32, name="ones_sb")
            nc.vector.memset(ones_sb, 1.0)
            eps_sb = pp.tile([128, 1], F32, name="eps_sb")
            nc.vector.memset(eps_sb, EPS)
            bo_sb = pp.tile([128, 8], F32, name="bo_sb")
            nc.sync.dma_start(out=bo_sb, in_=bo.rearrange("(k p) -> p k", p=128))
            b1_sb = pp.tile([128, 32], F32, name="b1_sb")
            nc.sync.dma_start(out=b1_sb, in_=b1.rearrange("(k p) -> p k", p=128))
            b2_sb = pp.tile([128, 8], F32, name="b2_sb")
            nc.sync.dma_start(out=b2_sb, in_=b2.rearrange("(k p) -> p k", p=128))

            x_tok = pp.tile([128, 4, D], F32, name="x_tok")
            nc.sync.dma_start(out=x_tok,
                              in_=x_own.rearrange("(t p) d -> p t d", p=128))

            lnT = pp.tile([128, 8, TOK], BF16, name="lnT")

            def layernorm_into_lnT(src_tile):
                # src_tile: [128, 4, 1024] f32 token-major residual stream
                for t in range(4):
                    xin = src_tile[:, t, :]
                    xg = xin.rearrange("p (g d) -> p g d", g=2)
                    stats = stp.tile([128, 2, 6], F32, name="stats")
                    for gsub in range(2):
                        nc.vector.bn_stats(out=stats[:, gsub, :],
                                           in_=xg[:, gsub, :])
                    mv = stp.tile([128, 2], F32, name="mv")
                    nc.vector.bn_aggr(out=mv, in_=stats)
                    rstd = stp.tile([128, 1], F32, name="rstd")
                    nc.scalar.activation(out=rstd, in_=mv[:, 1:2], func=AF.Sqrt,
                                         bias=eps_sb, scale=1.0)
                    nc.vector.reciprocal(out=rstd, in_=rstd)
                    xln = stg.tile([128, D], BF16, name="xln")
                    nc.vector.tensor_scalar(out=xln, in0=xin, scalar1=mv[:, 0:1],
                                            scalar2=rstd,
                                            op0=mybir.AluOpType.subtract,
                                            op1=mybir.AluOpType.mult)
                    for fb in range(8):
                        # alternate with idle psA slots: psT has only one
                        # bank, and LN runs while psA has no matmul work
                        if fb % 2 == 0:
                            pt = psA.tile([128, 128], BF16, name="acc")
                        else:
                            pt = psT.tile([128, 128], BF16, name="pt")
                        nc.tensor.transpose(pt, xln[:, fb * 128:(fb + 1) * 128],
                                            id_sb)
                        nc.vector.tensor_copy(
                            out=lnT[:, fb, t * 128:(t + 1) * 128], in_=pt)

            layernorm_into_lnT(x_tok)

            # ---- AllGather xln^T across all 8 cores ----
            ag_in = dp.tile([AG_IN], BF16, name="ag_in")
            ag_out = dp.tile([NCORES * AG_IN], BF16, name="ag_out",
                             addr_space="Shared")
            _agv = ag_in.rearrange("(a p t) -> a p t", a=8, p=128)
            for a in range(8):
                [nc.sync, nc.scalar][a % 2].dma_start(
                    out=_agv[a], in_=lnT[:, a, :])
            nc.gpsimd.collective_compute(
                "AllGather", mybir.AluOpType.bypass, replica_groups=rg,
                ins=[ag_in.opt()], outs=[ag_out.opt()])

            a2a_in = dp.tile([A2A_N], BF16, name="a2a_in")
            a2a_out = dp.tile([A2A_N], BF16, name="a2a_out")

            with tc.tile_pool(name="attnp", bufs=1) as ap_:
                xg_sb = ap_.tile([128, 8, NCORES * TOK], BF16, name="xg_sb")
                ag3 = ag_out.rearrange("(r a p t) -> r a p t", r=NCORES, a=8,
                                       p=128)
                _engs = [nc.gpsimd, nc.scalar]
                for r in range(NCORES):
                    _engs[r % 2].dma_start(
                        out=xg_sb[:, :, r * TOK:(r + 1) * TOK]
                        .rearrange("p a t -> p a t"),
                        in_=ag3[r].rearrange("a p t -> p a t"))

                wq_sb = ap_.tile([128, 8, B, 128], BF16, name="wq_sb")
                nc.sync.dma_start(out=wq_sb,
                                  in_=wq.rearrange("(k p) b m -> p k b m", p=128))
                wk_sb = ap_.tile([128, 8, B, 128], BF16, name="wk_sb")
                nc.sync.dma_start(out=wk_sb,
                                  in_=wk.rearrange("(k p) b m -> p k b m", p=128))
                wv_sb = ap_.tile([128, 8, B, 128], BF16, name="wv_sb")
                nc.sync.dma_start(out=wv_sb,
                                  in_=wv.rearrange("(k p) b m -> p k b m", p=128))
                bq_sb = ap_.tile([128, B], F32, name="bq_sb")
                nc.sync.dma_start(out=bq_sb, in_=bq.rearrange("b p -> p b"))
                bk_sb = ap_.tile([128, B], F32, name="bk_sb")
                nc.sync.dma_start(out=bk_sb, in_=bk.rearrange("b p -> p b"))
                wo_sb = pp.tile([128, 8, 1024], BF16, name="wo_sb")
                nc.sync.dma_start(
                    out=wo_sb, in_=wo.rearrange("(k p) m -> p k m", p=128))

                # ---- Q,K feature-major [128, 4096]; V token-major + ones ----
                qT = ap_.tile([128, B * S], BF16, name="qT")
                kT = ap_.tile([128, B * S], BF16, name="kT")
                for dst, wsb, bsb in ((qT, wq_sb, bq_sb), (kT, wk_sb, bk_sb)):
                    for b in range(B):
                        for tt in range(4):
                            c0 = b * S + tt * 512
                            acc = psA.tile([128, 512], F32, name="acc")
                            for kc in range(8):
                                nc.tensor.matmul(
                                    acc, lhsT=wsb[:, kc, b, :],
                                    rhs=xg_sb[:, kc, c0:c0 + 512],
                                    start=(kc == 0), stop=(kc == 7))
                            nc.vector.tensor_scalar_add(
                                out=dst[:, c0:c0 + 512], in0=acc,
                                scalar1=bsb[:, b:b + 1])

                vtok = ap_.tile([128, 32, 2, 65], BF16, name="vtok")
                nc.vector.memset(vtok[:, :, :, 64:65], 1.0)
                for b in range(B):
                    for tt in range(16):
                        acc = psA.tile([128, 128], F32, name="acc")
                        for kc in range(8):
                            nc.tensor.matmul(
                                acc,
                                lhsT=xg_sb[:, kc,
                                           b * S + tt * 128:b * S + tt * 128 + 128],
                                rhs=wv_sb[:, kc, b, :],
                                start=(kc == 0), stop=(kc == 7))
                        nc.vector.tensor_copy(
                            out=vtok[:, b * 16 + tt, :, 0:64],
                            in_=acc.rearrange("p (h c) -> p h c", h=2))

                # ---- causal attention: 2 heads x 2 batches, no max-sub ----
                attnT = ap_.tile([128, B, S], BF16, name="attnT")
                for b in range(B):
                    for hh in range(2):
                        hp = hh * 64
                        for q2 in range(8):
                            O = psO.tile([65, 256], F32, name="O")
                            nkt = 2 * q2 + 2
                            # kt chunks come in pairs (nkt is even): two
                            # score matmuls share one PSUM tile so a single
                            # exp covers 512 columns (ACT inst count halved)
                            for ktp in range(nkt // 2):
                                kt0 = 2 * ktp
                                sc = psS.tile([128, 2, 256], F32, name="sc")
                                for i in range(2):
                                    kt = kt0 + i
                                    nc.tensor.matmul(
                                        sc[:, i, :],
                                        lhsT=kT[hp:hp + 64,
                                                b * S + kt * 128:b * S + kt * 128 + 128],
                                        rhs=qT[hp:hp + 64,
                                               b * S + q2 * 256:b * S + q2 * 256 + 256],
                                        start=True, stop=True)
                                pt_ = ptp.tile([128, 2, 256], BF16, name="pt_")
                                nc.scalar.activation(out=pt_, in_=sc, func=AF.Exp)
                                if kt0 == nkt - 2:
                                    nc.vector.tensor_mul(out=pt_[:, 0, :],
                                                         in0=pt_[:, 0, :], in1=me_sb)
                                    nc.vector.tensor_mul(out=pt_[:, 1, :],
                                                         in0=pt_[:, 1, :], in1=mo_sb)
                                for i in range(2):
                                    kt = kt0 + i
                                    nc.tensor.matmul(O,
                                                     lhsT=vtok[:, b * 16 + kt, hh, :],
                                                     rhs=pt_[:, i, :],
                                                     start=(kt == 0),
                                                     stop=(kt == nkt - 1))
                            rc = rp.tile([1, 256], F32, name="rc")
                            nc.vector.reciprocal(out=rc, in_=O[64:65, :])
                            bc = psS.tile([128, 256], F32, name="bc")
                            nc.tensor.matmul(bc, lhsT=ones_sb, rhs=rc,
                                             start=True, stop=True)
                            nb = nbp.tile([64, 256], BF16, name="nb")
                            nc.vector.tensor_copy(out=nb, in_=O[0:64, :])
                            nc.vector.tensor_mul(
                                out=attnT[hp:hp + 64, b,
                                          q2 * 256:(q2 + 1) * 256],
                                in0=nb, in1=bc[0:64, :])

                # shard j of the A2A = my heads for batch j//4, tokens (j%4)
                nc.sync.dma_start(
                    out=a2a_in.rearrange("(s p t) -> p s t", s=8, p=128),
                    in_=attnT.rearrange("p b (jj t) -> p (b jj) t", jj=4))

            nc.gpsimd.collective_compute(
                "AllToAll", mybir.AluOpType.bypass, replica_groups=rg,
                ins=[a2a_in.opt()], outs=[a2a_out.opt()])

            with (
                tc.tile_pool(name="postp", bufs=1) as pc,
                tc.tile_pool(name="w1p", bufs=4) as w1p,
                tc.tile_pool(name="w2p", bufs=4) as w2p,
            ):
                af_sb = pc.tile([128, 8, TOK], BF16, name="af_sb")
                _af3 = a2a_out.rearrange("(i p t) -> i p t", i=8, p=128)
                for i in range(8):
                    [nc.gpsimd, nc.scalar][i % 2].dma_start(
                        out=af_sb[:, i, :], in_=_af3[i])

                # ---- wo projection + residual into x_tok (in place) ----
                yT = pc.tile([128, 8, TOK], BF16, name="yT")
                for fb in range(8):
                    acc = psA.tile([128, 512], F32, name="acc")
                    for kc in range(8):
                        nc.tensor.matmul(
                            acc, lhsT=wo_sb[:, kc, fb * 128:(fb + 1) * 128],
                            rhs=af_sb[:, kc, :],
                            start=(kc == 0), stop=(kc == 7))
                    nc.vector.tensor_scalar_add(out=yT[:, fb, :], in0=acc,
                                                scalar1=bo_sb[:, fb:fb + 1])
                for fb in range(8):
                    for t in range(4):
                        pt = psT.tile([128, 128], BF16, name="pt")
                        nc.tensor.transpose(pt, yT[:, fb, t * 128:(t + 1) * 128],
                                            id_sb)
                        nc.vector.tensor_add(
                            out=x_tok[:, t, fb * 128:(fb + 1) * 128],
                            in0=x_tok[:, t, fb * 128:(fb + 1) * 128], in1=pt)

                # ---- LN2 -> lnT (reused), FFN ----
                layernorm_into_lnT(x_tok)

                h1T = pc.tile([128, 32, TOK], BF16, name="h1T")
                for hbk in range(32):
                    w1t = w1p.tile([128, 8, 128], BF16, name="w1t")
                    nc.sync.dma_start(
                        out=w1t,
                        in_=w1[:, hbk * 128:(hbk + 1) * 128]
                        .rearrange("(k p) m -> p k m", p=128))
                    acc = psA.tile([128, 512], F32, name="acc")
                    for kc in range(8):
                        nc.tensor.matmul(acc, lhsT=w1t[:, kc, :],
                                         rhs=lnT[:, kc, :],
                                         start=(kc == 0), stop=(kc == 7))
                    nc.scalar.activation(out=h1T[:, hbk, :], in_=acc,
                                         func=AF.Gelu,
                                         bias=b1_sb[:, hbk:hbk + 1], scale=1.0)

                for fb in range(8):
                    acc = psA.tile([128, 512], F32, name="acc")
                    for hg in range(4):
                        w2t = w2p.tile([128, 8, 128], BF16, name="w2t")
                        nc.sync.dma_start(
                            out=w2t,
                            in_=w2[hg * 1024:(hg + 1) * 1024,
                                   fb * 128:(fb + 1) * 128]
                            .rearrange("(k p) m -> p k m", p=128))
                        for kc in range(8):
                            nc.tensor.matmul(acc, lhsT=w2t[:, kc, :],
                                             rhs=h1T[:, hg * 8 + kc, :],
                                             start=(hg == 0 and kc == 0),
                                             stop=(hg == 3 and kc == 7))
                    y2T = stg.tile([128, TOK], BF16, name="y2T")
                    nc.vector.tensor_scalar_add(out=y2T, in0=acc,
                                                scalar1=b2_sb[:, fb:fb + 1])
                    for t in range(4):
                        pt = psT.tile([128, 128], BF16, name="pt")
                        nc.tensor.transpose(pt, y2T[:, t * 128:(t + 1) * 128],
                                            id_sb)
                        nc.vector.tensor_add(
                            out=x_tok[:, t, fb * 128:(fb + 1) * 128],
                            in0=x_tok[:, t, fb * 128:(fb + 1) * 128], in1=pt)

                nc.sync.dma_start(out=out.rearrange("(t p) d -> p t d", p=128),
                                  in_=x_tok)
    nc.compile()
    return nc


_NC_CACHE = {}


def _get_nc():
    if "nc" not in _NC_CACHE:
        _NC_CACHE["nc"] = build_nc()
    return _NC_CACHE["nc"]


def _prep_in_maps(x, ln1_g, ln1_b, wq, bq, wk, bk, wv, bv, wo, bo,
                  ln2_g, ln2_b, w1, b1, w2, b2):
    bf16 = ml_dtypes.bfloat16
    f32 = np.float32
    x = np.asarray(x, f32)
    DK = 64
    sc = 1.0 / np.sqrt(DK)
    ln1_g = np.asarray(ln1_g, f32)
    ln1_b = np.asarray(ln1_b, f32)
    ln2_g = np.asarray(ln2_g, f32)
    ln2_b = np.asarray(ln2_b, f32)
    wq = np.asarray(wq, f32)
    wk = np.asarray(wk, f32)
    wv = np.asarray(wv, f32)
    wo_np = np.asarray(wo, f32)
    w1 = np.asarray(w1, f32)
    w2 = np.asarray(w2, f32)

    wq_f = (ln1_g[:, None] * wq * sc).astype(bf16)
    bq_f = ((ln1_b @ wq + np.asarray(bq, f32)) * sc).astype(f32)
    wk_f = (ln1_g[:, None] * wk).astype(bf16)
    bk_f = (ln1_b @ wk + np.asarray(bk, f32)).astype(f32)
    wv_f = (ln1_g[:, None] * wv).astype(bf16)
    bv_f = (ln1_b @ wv + np.asarray(bv, f32)).astype(f32)
    bo_f = (np.asarray(bo, f32) + bv_f @ wo_np).astype(f32)
    wo_f = wo_np.astype(bf16)
    w1_f = (ln2_g[:, None] * w1).astype(bf16)
    b1_f = (ln2_b @ w1 + np.asarray(b1, f32)).astype(f32)
    w2_f = w2.astype(bf16)
    b2_f = np.asarray(b2, f32)

    tri = np.triu(np.ones((128, 128), f32))
    me_np = np.concatenate([tri, np.ones((128, 128), f32)], 1).astype(bf16)
    mo_np = np.concatenate([np.zeros((128, 128), f32), tri], 1).astype(bf16)
    id128 = np.eye(128, dtype=f32).astype(bf16)

    in_maps = []
    for core in range(NCORES):
        g, l = divmod(core, GRP)

        def hsel(b, j=core):
            m = (j + 4 * b) % 8
            return slice(m * 128, (m + 1) * 128)

        wo_perm = np.concatenate(
            [wo_f[((i + 4 * g) % 8) * 128:((i + 4 * g) % 8) * 128 + 128, :]
             for i in range(8)], axis=0)
        in_maps.append({
            "x_own": np.ascontiguousarray(x[g, l * TOK:(l + 1) * TOK, :]),
            "wq": np.ascontiguousarray(
                np.stack([wq_f[:, hsel(b)] for b in range(B)], axis=1)),
            "wk": np.ascontiguousarray(
                np.stack([wk_f[:, hsel(b)] for b in range(B)], axis=1)),
            "wv": np.ascontiguousarray(
                np.stack([wv_f[:, hsel(b)] for b in range(B)], axis=1)),
            "bq": np.ascontiguousarray(
                np.stack([bq_f[hsel(b)] for b in range(B)])),
            "bk": np.ascontiguousarray(
                np.stack([bk_f[hsel(b)] for b in range(B)])),
            "wo": np.ascontiguousarray(wo_perm), "bo": bo_f,
            "w1": w1_f, "b1": b1_f, "w2": w2_f, "b2": b2_f,
            "id128": id128, "me": me_np, "mo": mo_np,
        })
    return in_maps


def kernel(**inputs):
    nc = _get_nc()
    in_maps = _prep_in_maps(**inputs)
    res = run_bass_kernel_spmd(nc, in_maps, core_ids=list(range(NCORES)))
    full = np.empty((B, S, D), np.float32)
    for core in range(NCORES):
        g, l = divmod(core, GRP)
        full[g, l * TOK:(l + 1) * TOK, :] = res.results[core]["out"]
    return full



# revision 10
# speedup vs baseline: 1.1113x; 1.1113x over previous
"""Trainium2 Bass kernel: dense transformer block (LN1-attn-LN2-FFN, causal, 16 heads).

Sharding (8 NeuronCores, SPMD one graph):
  - core j: token-parallel for LN/FFN/residual: owns tokens [512l, 512(l+1))
    of batch g, where g, l = divmod(j, 4)
  - attention head-parallel with cyclic head-batch assignment: core j computes
    head pair {2m, 2m+1}, m = (j + 4b) % 8, for EACH batch b over the full
    2048-token sequence. Uniform causal loop structure on every core; all
    per-core variation (which heads / which tokens) lives in the input data.
  - comm: 8-core AllGather of LN1^T output (QKV sees all tokens), 8-core
    AllToAll of normalized attention^T (head-shard -> token-shard). The
    receive-side head permutation is folded into host-permuted wo rows.
  - matmuls bf16 (f32 accumulate); residual stream f32; softmax without
    max-subtraction (scores are O(1) for this problem scale).
  - LN gains/biases, 1/sqrt(dk), and bv are folded into weights host-side.
"""

import numpy as np
import ml_dtypes

import concourse.bass as bass
import concourse.tile as tile
from concourse import bacc, mybir
from concourse.bass_utils import run_bass_kernel_spmd

F32 = mybir.dt.float32
BF16 = mybir.dt.bfloat16
AF = mybir.ActivationFunctionType

D = 1024
DFF = 4096
B = 2
S = 2048
NCORES = 8
GRP = 4
TOK = 512        # tokens per core (FFN/LN shard)
EPS = 1e-5

AG_IN = D * TOK          # bf16 elems contributed to xln AllGather
A2A_N = NCORES * 128 * TOK   # total elems in the 8-way AllToAll


def build_nc():
    nc = bacc.Bacc("TRN2", target_bir_lowering=False, debug=False,
                   num_devices=NCORES)

    x_own = nc.dram_tensor("x_own", [TOK, D], F32, kind="ExternalInput").ap()
    wq = nc.dram_tensor("wq", [D, B, 128], BF16, kind="ExternalInput").ap()
    wk = nc.dram_tensor("wk", [D, B, 128], BF16, kind="ExternalInput").ap()
    wv = nc.dram_tensor("wv", [D, B, 128], BF16, kind="ExternalInput").ap()
    bq = nc.dram_tensor("bq", [B, 128], F32, kind="ExternalInput").ap()
    bk = nc.dram_tensor("bk", [B, 128], F32, kind="ExternalInput").ap()
    wo = nc.dram_tensor("wo", [D, D], BF16, kind="ExternalInput").ap()
    bo = nc.dram_tensor("bo", [D], F32, kind="ExternalInput").ap()
    w1 = nc.dram_tensor("w1", [D, DFF], BF16, kind="ExternalInput").ap()
    b1 = nc.dram_tensor("b1", [DFF], F32, kind="ExternalInput").ap()
    w2 = nc.dram_tensor("w2", [DFF, D], BF16, kind="ExternalInput").ap()
    b2 = nc.dram_tensor("b2", [D], F32, kind="ExternalInput").ap()
    id128 = nc.dram_tensor("id128", [128, 128], BF16, kind="ExternalInput").ap()
    masks = nc.dram_tensor("masks", [128, 4, 512], BF16,
                           kind="ExternalInput").ap()
    out = nc.dram_tensor("out", [TOK, D], F32, kind="ExternalOutput").ap()

    rg = [list(range(NCORES))]

    with tile.TileContext(nc) as tc:
        with (
            tc.tile_pool(name="persist", bufs=1) as pp,
            tc.tile_pool(name="stage", bufs=2) as stg,
            tc.tile_pool(name="stats", bufs=4) as stp,
            tc.tile_pool(name="ptp", bufs=6) as ptp,
            tc.tile_pool(name="rp", bufs=2) as rp,
            tc.tile_pool(name="psA", bufs=2, space="PSUM") as psA,
            tc.tile_pool(name="psS", bufs=2, space="PSUM") as psS,
            tc.tile_pool(name="psO", bufs=1, space="PSUM") as psO,
            tc.tile_pool(name="dram", bufs=1, space="DRAM") as dp,
        ):
            # ---- constants ----
            id_sb = pp.tile([128, 128], BF16, name="id_sb")
            nc.sync.dma_start(out=id_sb, in_=id128)
            mask_sb = pp.tile([128, 4, 512], BF16, name="mask_sb")
            nc.sync.dma_start(out=mask_sb, in_=masks)
            eps_sb = pp.tile([128, 1], F32, name="eps_sb")
            nc.vector.memset(eps_sb, EPS)
            bo_sb = pp.tile([128, 8], F32, name="bo_sb")
            nc.sync.dma_start(out=bo_sb, in_=bo.rearrange("(k p) -> p k", p=128))
            b1_sb = pp.tile([128, 32], F32, name="b1_sb")
            nc.sync.dma_start(out=b1_sb, in_=b1.rearrange("(k p) -> p k", p=128))
            b2_sb = pp.tile([128, 8], F32, name="b2_sb")
            nc.sync.dma_start(out=b2_sb, in_=b2.rearrange("(k p) -> p k", p=128))

            x_tok = pp.tile([128, 4, D], F32, name="x_tok")
            nc.sync.dma_start(out=x_tok,
                              in_=x_own.rearrange("(t p) d -> p t d", p=128))

            lnT = pp.tile([128, 8, TOK], BF16, name="lnT")

            def layernorm_into_lnT(src_tile):
                # src_tile: [128, 4, 1024] f32 token-major residual stream
                for t in range(4):
                    xin = src_tile[:, t, :]
                    xg = xin.rearrange("p (g d) -> p g d", g=2)
                    stats = stp.tile([128, 2, 6], F32, name="stats")
                    for gsub in range(2):
                        nc.vector.bn_stats(out=stats[:, gsub, :],
                                           in_=xg[:, gsub, :])
                    mv = stp.tile([128, 2], F32, name="mv")
                    nc.vector.bn_aggr(out=mv, in_=stats)
                    rstd = stp.tile([128, 1], F32, name="rstd")
                    nc.scalar.activation(out=rstd, in_=mv[:, 1:2], func=AF.Sqrt,
                                         bias=eps_sb, scale=1.0)
                    nc.vector.reciprocal(out=rstd, in_=rstd)
                    xln = stg.tile([128, D], BF16, name="xln")
                    nc.vector.tensor_scalar(out=xln, in0=xin, scalar1=mv[:, 0:1],
                                            scalar2=rstd,
                                            op0=mybir.AluOpType.subtract,
                                            op1=mybir.AluOpType.mult)
                    for fb in range(8):
                        pt = psA.tile([128, 128], BF16, name="acc")
                        nc.tensor.transpose(pt, xln[:, fb * 128:(fb + 1) * 128],
                                            id_sb)
                        nc.vector.tensor_copy(
                            out=lnT[:, fb, t * 128:(t + 1) * 128], in_=pt)

            layernorm_into_lnT(x_tok)

            # ---- AllGather xln^T across all 8 cores ----
            ag_in = dp.tile([AG_IN], BF16, name="ag_in")
            ag_out = dp.tile([NCORES * AG_IN], BF16, name="ag_out",
                             addr_space="Shared")
            _agv = ag_in.rearrange("(a p t) -> a p t", a=8, p=128)
            for a in range(8):
                [nc.sync, nc.scalar][a % 2].dma_start(
                    out=_agv[a], in_=lnT[:, a, :])
            nc.gpsimd.collective_compute(
                "AllGather", mybir.AluOpType.bypass, replica_groups=rg,
                ins=[ag_in.opt()], outs=[ag_out.opt()])

            a2a_in = dp.tile([A2A_N], BF16, name="a2a_in")
            a2a_out = dp.tile([A2A_N], BF16, name="a2a_out")

            with tc.tile_pool(name="attnp", bufs=1) as ap_:
                xg_sb = ap_.tile([128, 8, NCORES * TOK], BF16, name="xg_sb")
                ag3 = ag_out.rearrange("(r a p t) -> r a p t", r=NCORES, a=8,
                                       p=128)
                _engs = [nc.gpsimd, nc.scalar]
                for r in range(NCORES):
                    _engs[r % 2].dma_start(
                        out=xg_sb[:, :, r * TOK:(r + 1) * TOK]
                        .rearrange("p a t -> p a t"),
                        in_=ag3[r].rearrange("a p t -> p a t"))

                wq_sb = ap_.tile([128, 8, B, 128], BF16, name="wq_sb")
                nc.sync.dma_start(out=wq_sb,
                                  in_=wq.rearrange("(k p) b m -> p k b m", p=128))
                wk_sb = ap_.tile([128, 8, B, 128], BF16, name="wk_sb")
                nc.sync.dma_start(out=wk_sb,
                                  in_=wk.rearrange("(k p) b m -> p k b m", p=128))
                wv_sb = ap_.tile([128, 8, B, 128], BF16, name="wv_sb")
                nc.sync.dma_start(out=wv_sb,
                                  in_=wv.rearrange("(k p) b m -> p k b m", p=128))
                bq_sb = ap_.tile([128, B], F32, name="bq_sb")
                nc.sync.dma_start(out=bq_sb, in_=bq.rearrange("b p -> p b"))
                bk_sb = ap_.tile([128, B], F32, name="bk_sb")
                nc.sync.dma_start(out=bk_sb, in_=bk.rearrange("b p -> p b"))
                wo_sb = pp.tile([128, 8, 1024], BF16, name="wo_sb")
                nc.sync.dma_start(
                    out=wo_sb, in_=wo.rearrange("(k p) m -> p k m", p=128))

                # ---- Q,K feature-major [128, 4096]; V token-major + ones ----
                qT = ap_.tile([128, B * S], BF16, name="qT")
                kT = ap_.tile([128, B * S], BF16, name="kT")
                for dst, wsb, bsb in ((qT, wq_sb, bq_sb), (kT, wk_sb, bk_sb)):
                    for b in range(B):
                        for tt in range(4):
                            c0 = b * S + tt * 512
                            acc = psA.tile([128, 512], F32, name="acc")
                            for kc in range(8):
                                nc.tensor.matmul(
                                    acc, lhsT=wsb[:, kc, b, :],
                                    rhs=xg_sb[:, kc, c0:c0 + 512],
                                    start=(kc == 0), stop=(kc == 7))
                            nc.vector.tensor_scalar_add(
                                out=dst[:, c0:c0 + 512], in0=acc,
                                scalar1=bsb[:, b:b + 1])

                # V token-major; cols 64:128 are ones so the AV matmul also
                # emits the softmax denominator replicated on rows 64:128
                vtok = ap_.tile([128, 32, 2, 128], BF16, name="vtok")
                nc.vector.memset(vtok[:, :, :, 64:128], 1.0)
                for b in range(B):
                    for tt in range(16):
                        acc = psA.tile([128, 128], F32, name="acc")
                        for kc in range(8):
                            nc.tensor.matmul(
                                acc,
                                lhsT=xg_sb[:, kc,
                                           b * S + tt * 128:b * S + tt * 128 + 128],
                                rhs=wv_sb[:, kc, b, :],
                                start=(kc == 0), stop=(kc == 7))
                        nc.vector.tensor_copy(
                            out=vtok[:, b * 16 + tt, :, 0:64],
                            in_=acc.rearrange("p (h c) -> p h c", h=2))

                # ---- causal attention: 2 heads x 2 batches, no max-sub.
                # q blocks of 512, k blocks of 128; the two heads' score
                # matmuls are row-packed (K=64 at PE rows 0/64) and run
                # concurrently in the PE array.
                attnT = ap_.tile([128, B, S], BF16, name="attnT")
                for b in range(B):
                    for q4 in range(4):
                        c0 = b * S + q4 * 512
                        O = [psO.tile([128, 512], F32, name=f"O{hh}")
                             for hh in range(2)]
                        nkt = 4 * q4 + 4
                        for kt in range(nkt):
                            k0 = b * S + kt * 128
                            sc = psS.tile([128, 2, 512], F32, name="sc")
                            for hh in range(2):
                                hp = hh * 64
                                nc.tensor.matmul(
                                    sc[:, hh, :],
                                    lhsT=kT[hp:hp + 64, k0:k0 + 128],
                                    rhs=qT[hp:hp + 64, c0:c0 + 512],
                                    start=True, stop=True)
                            pt_ = ptp.tile([128, 2, 512], BF16, name="pt_")
                            nc.scalar.activation(out=pt_, in_=sc, func=AF.Exp)
                            m = kt - 4 * q4
                            if m >= 0:
                                for hh in range(2):
                                    nc.vector.tensor_mul(
                                        out=pt_[:, hh, :], in0=pt_[:, hh, :],
                                        in1=mask_sb[:, m, :])
                            for hh in range(2):
                                nc.tensor.matmul(
                                    O[hh], lhsT=vtok[:, b * 16 + kt, hh, :],
                                    rhs=pt_[:, hh, :],
                                    start=(kt == 0), stop=(kt == nkt - 1))
                        for hh in range(2):
                            rec = rp.tile([64, 512], F32, name="rec")
                            nc.vector.reciprocal(out=rec, in_=O[hh][64:128, :])
                            nc.vector.tensor_mul(
                                out=attnT[hh * 64:hh * 64 + 64, b,
                                          q4 * 512:(q4 + 1) * 512],
                                in0=O[hh][0:64, :], in1=rec)

                # shard j of the A2A = my heads for batch j//4, tokens (j%4)
                nc.sync.dma_start(
                    out=a2a_in.rearrange("(s p t) -> p s t", s=8, p=128),
                    in_=attnT.rearrange("p b (jj t) -> p (b jj) t", jj=4))

            nc.gpsimd.collective_compute(
                "AllToAll", mybir.AluOpType.bypass, replica_groups=rg,
                ins=[a2a_in.opt()], outs=[a2a_out.opt()])

            with (
                tc.tile_pool(name="postp", bufs=1) as pc,
                tc.tile_pool(name="w1p", bufs=4) as w1p,
                tc.tile_pool(name="w2p", bufs=4) as w2p,
            ):
                af_sb = pc.tile([128, 8, TOK], BF16, name="af_sb")
                _af3 = a2a_out.rearrange("(i p t) -> i p t", i=8, p=128)
                for i in range(8):
                    [nc.gpsimd, nc.scalar][i % 2].dma_start(
                        out=af_sb[:, i, :], in_=_af3[i])

                # ---- wo projection + residual into x_tok (in place) ----
                yT = pc.tile([128, 8, TOK], BF16, name="yT")
                for fb in range(8):
                    acc = psA.tile([128, 512], F32, name="acc")
                    for kc in range(8):
                        nc.tensor.matmul(
                            acc, lhsT=wo_sb[:, kc, fb * 128:(fb + 1) * 128],
                            rhs=af_sb[:, kc, :],
                            start=(kc == 0), stop=(kc == 7))
                    nc.vector.tensor_scalar_add(out=yT[:, fb, :], in0=acc,
                                                scalar1=bo_sb[:, fb:fb + 1])
                for fb in range(8):
                    for t in range(4):
                        pt = psA.tile([128, 128], BF16, name="acc")
                        nc.tensor.transpose(pt, yT[:, fb, t * 128:(t + 1) * 128],
                                            id_sb)
                        nc.vector.tensor_add(
                            out=x_tok[:, t, fb * 128:(fb + 1) * 128],
                            in0=x_tok[:, t, fb * 128:(fb + 1) * 128], in1=pt)

                # ---- LN2 -> lnT (reused), FFN ----
                layernorm_into_lnT(x_tok)

                h1T = pc.tile([128, 32, TOK], BF16, name="h1T")
                for hbk in range(32):
                    w1t = w1p.tile([128, 8, 128], BF16, name="w1t")
                    nc.sync.dma_start(
                        out=w1t,
                        in_=w1[:, hbk * 128:(hbk + 1) * 128]
                        .rearrange("(k p) m -> p k m", p=128))
                    acc = psA.tile([128, 512], F32, name="acc")
                    for kc in range(8):
                        nc.tensor.matmul(acc, lhsT=w1t[:, kc, :],
                                         rhs=lnT[:, kc, :],
                                         start=(kc == 0), stop=(kc == 7))
                    nc.scalar.activation(out=h1T[:, hbk, :], in_=acc,
                                         func=AF.Gelu,
                                         bias=b1_sb[:, hbk:hbk + 1], scale=1.0)

                for fb in range(8):
                    acc = psA.tile([128, 512], F32, name="acc")
                    for hg in range(4):
                        w2t = w2p.tile([128, 8, 128], BF16, name="w2t")
                        nc.sync.dma_start(
                            out=w2t,
                            in_=w2[hg * 1024:(hg + 1) * 1024,
                                   fb * 128:(fb + 1) * 128]
                            .rearrange("(k p) m -> p k m", p=128))
                        for kc in range(8):
                            nc.tensor.matmul(acc, lhsT=w2t[:, kc, :],
                                             rhs=h1T[:, hg * 8 + kc, :],
                                             start=(hg == 0 and kc == 0),
                                             stop=(hg == 3 and kc == 7))
                    y2T = stg.tile([128, TOK], BF16, name="y2T")
                    nc.vector.tensor_scalar_add(out=y2T, in0=acc,
                                                scalar1=b2_sb[:, fb:fb + 1])
                    for t in range(4):
                        pt = psA.tile([128, 128], BF16, name="acc")
                        nc.tensor.transpose(pt, y2T[:, t * 128:(t + 1) * 128],
                                            id_sb)
                        nc.vector.tensor_add(
                            out=x_tok[:, t, fb * 128:(fb + 1) * 128],
                            in0=x_tok[:, t, fb * 128:(fb + 1) * 128], in1=pt)

                nc.sync.dma_start(out=out.rearrange("(t p) d -> p t d", p=128),
                                  in_=x_tok)
    nc.compile()
    return nc


_NC_CACHE = {}


def _get_nc():
    if "nc" not in _NC_CACHE:
        _NC_CACHE["nc"] = build_nc()
    return _NC_CACHE["nc"]


def _prep_in_maps(x, ln1_g, ln1_b, wq, bq, wk, bk, wv, bv, wo, bo,
                  ln2_g, ln2_b, w1, b1, w2, b2):
    bf16 = ml_dtypes.bfloat16
    f32 = np.float32
    x = np.asarray(x, f32)
    DK = 64
    sc = 1.0 / np.sqrt(DK)
    ln1_g = np.asarray(ln1_g, f32)
    ln1_b = np.asarray(ln1_b, f32)
    ln2_g = np.asarray(ln2_g, f32)
    ln2_b = np.asarray(ln2_b, f32)
    wq = np.asarray(wq, f32)
    wk = np.asarray(wk, f32)
    wv = np.asarray(wv, f32)
    wo_np = np.asarray(wo, f32)
    w1 = np.asarray(w1, f32)
    w2 = np.asarray(w2, f32)

    wq_f = (ln1_g[:, None] * wq * sc).astype(bf16)
    bq_f = ((ln1_b @ wq + np.asarray(bq, f32)) * sc).astype(f32)
    wk_f = (ln1_g[:, None] * wk).astype(bf16)
    bk_f = (ln1_b @ wk + np.asarray(bk, f32)).astype(f32)
    wv_f = (ln1_g[:, None] * wv).astype(bf16)
    bv_f = (ln1_b @ wv + np.asarray(bv, f32)).astype(f32)
    bo_f = (np.asarray(bo, f32) + bv_f @ wo_np).astype(f32)
    wo_f = wo_np.astype(bf16)
    w1_f = (ln2_g[:, None] * w1).astype(bf16)
    b1_f = (ln2_b @ w1 + np.asarray(b1, f32)).astype(f32)
    w2_f = w2.astype(bf16)
    b2_f = np.asarray(b2, f32)

    # masks[i, m, j] = 1 iff q position j (within a 512 block) >= k position
    # i + 128*m (k block m of the diagonal 512-token region)
    ii = np.arange(128)[:, None, None]
    mm_ = np.arange(4)[None, :, None]
    jj = np.arange(512)[None, None, :]
    masks_np = (jj >= ii + 128 * mm_).astype(f32).astype(bf16)
    id128 = np.eye(128, dtype=f32).astype(bf16)

    in_maps = []
    for core in range(NCORES):
        g, l = divmod(core, GRP)

        def hsel(b, j=core):
            m = (j + 4 * b) % 8
            return slice(m * 128, (m + 1) * 128)

        wo_perm = np.concatenate(
            [wo_f[((i + 4 * g) % 8) * 128:((i + 4 * g) % 8) * 128 + 128, :]
             for i in range(8)], axis=0)
        in_maps.append({
            "x_own": np.ascontiguousarray(x[g, l * TOK:(l + 1) * TOK, :]),
            "wq": np.ascontiguousarray(
                np.stack([wq_f[:, hsel(b)] for b in range(B)], axis=1)),
            "wk": np.ascontiguousarray(
                np.stack([wk_f[:, hsel(b)] for b in range(B)], axis=1)),
            "wv": np.ascontiguousarray(
                np.stack([wv_f[:, hsel(b)] for b in range(B)], axis=1)),
            "bq": np.ascontiguousarray(
                np.stack([bq_f[hsel(b)] for b in range(B)])),
            "bk": np.ascontiguousarray(
                np.stack([bk_f[hsel(b)] for b in range(B)])),
            "wo": np.ascontiguousarray(wo_perm), "bo": bo_f,
            "w1": w1_f, "b1": b1_f, "w2": w2_f, "b2": b2_f,
            "id128": id128, "masks": masks_np,
        })
    return in_maps


def kernel(**inputs):
    nc = _get_nc()
    in_maps = _prep_in_maps(**inputs)
    res = run_bass_kernel_spmd(nc, in_maps, core_ids=list(range(NCORES)))
    full = np.empty((B, S, D), np.float32)
    for core in range(NCORES):
        g, l = divmod(core, GRP)
        full[g, l * TOK:(l + 1) * TOK, :] = res.results[core]["out"]
    return full



# revision 12
# speedup vs baseline: 1.1254x; 1.0128x over previous
"""Trainium2 Bass kernel: dense transformer block (LN1-attn-LN2-FFN, causal, 16 heads).

Sharding (8 NeuronCores, SPMD one graph):
  - core j: token-parallel for LN/FFN/residual: owns tokens [512l, 512(l+1))
    of batch g, where g, l = divmod(j, 4)
  - attention head-parallel with cyclic head-batch assignment: core j computes
    head pair {2m, 2m+1}, m = (j + 4b) % 8, for EACH batch b over the full
    2048-token sequence. Uniform causal loop structure on every core; all
    per-core variation (which heads / which tokens) lives in the input data.
  - comm: 8-core AllGather of LN1^T output (QKV sees all tokens), 8-core
    AllToAll of normalized attention^T (head-shard -> token-shard). The
    receive-side head permutation is folded into host-permuted wo rows.
  - matmuls bf16 (f32 accumulate); residual stream f32; softmax without
    max-subtraction (scores are O(1) for this problem scale).
  - LN gains/biases, 1/sqrt(dk), and bv are folded into weights host-side.
"""

import numpy as np
import ml_dtypes

import concourse.bass as bass
import concourse.tile as tile
from concourse import bacc, mybir
from concourse.bass_utils import run_bass_kernel_spmd

F32 = mybir.dt.float32
BF16 = mybir.dt.bfloat16
AF = mybir.ActivationFunctionType

D = 1024
DFF = 4096
B = 2
S = 2048
NCORES = 8
GRP = 4
TOK = 512        # tokens per core (FFN/LN shard)
EPS = 1e-5

AG_IN = D * TOK          # bf16 elems contributed to xln AllGather
A2A_N = NCORES * 128 * TOK   # total elems in the 8-way AllToAll


def build_nc():
    nc = bacc.Bacc("TRN2", target_bir_lowering=False, debug=False,
                   num_devices=NCORES)

    x_own = nc.dram_tensor("x_own", [TOK, D], F32, kind="ExternalInput").ap()
    wq = nc.dram_tensor("wq", [D, B, 128], BF16, kind="ExternalInput").ap()
    wk = nc.dram_tensor("wk", [D, B, 128], BF16, kind="ExternalInput").ap()
    wv = nc.dram_tensor("wv", [D, B, 128], BF16, kind="ExternalInput").ap()
    bq = nc.dram_tensor("bq", [B, 128], F32, kind="ExternalInput").ap()
    bk = nc.dram_tensor("bk", [B, 128], F32, kind="ExternalInput").ap()
    wo = nc.dram_tensor("wo", [D, D], BF16, kind="ExternalInput").ap()
    bo = nc.dram_tensor("bo", [D], F32, kind="ExternalInput").ap()
    w1 = nc.dram_tensor("w1", [D, DFF], BF16, kind="ExternalInput").ap()
    b1 = nc.dram_tensor("b1", [DFF], F32, kind="ExternalInput").ap()
    w2 = nc.dram_tensor("w2", [DFF, D], BF16, kind="ExternalInput").ap()
    b2 = nc.dram_tensor("b2", [D], F32, kind="ExternalInput").ap()
    id128 = nc.dram_tensor("id128", [128, 128], BF16, kind="ExternalInput").ap()
    masks = nc.dram_tensor("masks", [128, 4, 512], BF16,
                           kind="ExternalInput").ap()
    out = nc.dram_tensor("out", [TOK, D], F32, kind="ExternalOutput").ap()

    rg = [list(range(NCORES))]

    with tile.TileContext(nc) as tc:
        with (
            tc.tile_pool(name="persist", bufs=1) as pp,
            tc.tile_pool(name="stage", bufs=2) as stg,
            tc.tile_pool(name="stats", bufs=4) as stp,
            tc.tile_pool(name="ptp", bufs=6) as ptp,
            tc.tile_pool(name="rp", bufs=2) as rp,
            tc.tile_pool(name="psA", bufs=2, space="PSUM") as psA,
            tc.tile_pool(name="psS", bufs=2, space="PSUM") as psS,
            tc.tile_pool(name="psO", bufs=1, space="PSUM") as psO,
            tc.tile_pool(name="dram", bufs=1, space="DRAM") as dp,
        ):
            # ---- constants ----
            id_sb = pp.tile([128, 128], BF16, name="id_sb")
            nc.sync.dma_start(out=id_sb, in_=id128)
            mask_sb = pp.tile([128, 4, 512], BF16, name="mask_sb")
            nc.sync.dma_start(out=mask_sb, in_=masks)
            eps_sb = pp.tile([128, 1], F32, name="eps_sb")
            nc.vector.memset(eps_sb, EPS)
            bo_sb = pp.tile([128, 8], F32, name="bo_sb")
            nc.sync.dma_start(out=bo_sb, in_=bo.rearrange("(k p) -> p k", p=128))
            b1_sb = pp.tile([128, 32], F32, name="b1_sb")
            nc.sync.dma_start(out=b1_sb, in_=b1.rearrange("(k p) -> p k", p=128))
            b2_sb = pp.tile([128, 8], F32, name="b2_sb")
            nc.sync.dma_start(out=b2_sb, in_=b2.rearrange("(k p) -> p k", p=128))

            x_tok = pp.tile([128, 4, D], F32, name="x_tok")
            nc.sync.dma_start(out=x_tok,
                              in_=x_own.rearrange("(t p) d -> p t d", p=128))

            lnT = pp.tile([128, 8, TOK], BF16, name="lnT")

            def layernorm_into_lnT(src_tile):
                # src_tile: [128, 4, 1024] f32 token-major residual stream
                for t in range(4):
                    xin = src_tile[:, t, :]
                    xg = xin.rearrange("p (g d) -> p g d", g=2)
                    stats = stp.tile([128, 2, 6], F32, name="stats")
                    for gsub in range(2):
                        nc.vector.bn_stats(out=stats[:, gsub, :],
                                           in_=xg[:, gsub, :])
                    mv = stp.tile([128, 2], F32, name="mv")
                    nc.vector.bn_aggr(out=mv, in_=stats)
                    rstd = stp.tile([128, 1], F32, name="rstd")
                    nc.scalar.activation(out=rstd, in_=mv[:, 1:2], func=AF.Sqrt,
                                         bias=eps_sb, scale=1.0)
                    nc.vector.reciprocal(out=rstd, in_=rstd)
                    xln = stg.tile([128, D], BF16, name="xln")
                    nc.vector.tensor_scalar(out=xln, in0=xin, scalar1=mv[:, 0:1],
                                            scalar2=rstd,
                                            op0=mybir.AluOpType.subtract,
                                            op1=mybir.AluOpType.mult)
                    for fb in range(8):
                        pt = psA.tile([128, 128], BF16, name="acc")
                        nc.tensor.transpose(pt, xln[:, fb * 128:(fb + 1) * 128],
                                            id_sb)
                        nc.vector.tensor_copy(
                            out=lnT[:, fb, t * 128:(t + 1) * 128], in_=pt)

            layernorm_into_lnT(x_tok)

            # ---- AllGather xln^T across all 8 cores ----
            ag_in = dp.tile([AG_IN], BF16, name="ag_in")
            ag_out = dp.tile([NCORES * AG_IN], BF16, name="ag_out",
                             addr_space="Shared")
            _agv = ag_in.rearrange("(a p t) -> a p t", a=8, p=128)
            for a in range(8):
                [nc.sync, nc.scalar][a % 2].dma_start(
                    out=_agv[a], in_=lnT[:, a, :])
            nc.gpsimd.collective_compute(
                "AllGather", mybir.AluOpType.bypass, replica_groups=rg,
                ins=[ag_in.opt()], outs=[ag_out.opt()])

            a2a_in = dp.tile([A2A_N], BF16, name="a2a_in")
            a2a_out = dp.tile([A2A_N], BF16, name="a2a_out")

            with tc.tile_pool(name="attnp", bufs=1) as ap_:
                xg_sb = ap_.tile([128, 8, NCORES * TOK], BF16, name="xg_sb")
                ag3 = ag_out.rearrange("(r a p t) -> r a p t", r=NCORES, a=8,
                                       p=128)
                _engs = [nc.gpsimd, nc.scalar]
                for r in range(NCORES):
                    _engs[r % 2].dma_start(
                        out=xg_sb[:, :, r * TOK:(r + 1) * TOK]
                        .rearrange("p a t -> p a t"),
                        in_=ag3[r].rearrange("a p t -> p a t"))

                wq_sb = ap_.tile([128, 8, B, 128], BF16, name="wq_sb")
                nc.sync.dma_start(out=wq_sb,
                                  in_=wq.rearrange("(k p) b m -> p k b m", p=128))
                wk_sb = ap_.tile([128, 8, B, 128], BF16, name="wk_sb")
                nc.sync.dma_start(out=wk_sb,
                                  in_=wk.rearrange("(k p) b m -> p k b m", p=128))
                wv_sb = ap_.tile([128, 8, B, 128], BF16, name="wv_sb")
                nc.sync.dma_start(out=wv_sb,
                                  in_=wv.rearrange("(k p) b m -> p k b m", p=128))
                bq_sb = ap_.tile([128, B], F32, name="bq_sb")
                nc.sync.dma_start(out=bq_sb, in_=bq.rearrange("b p -> p b"))
                bk_sb = ap_.tile([128, B], F32, name="bk_sb")
                nc.sync.dma_start(out=bk_sb, in_=bk.rearrange("b p -> p b"))
                wo_sb = pp.tile([128, 8, 1024], BF16, name="wo_sb")
                nc.sync.dma_start(
                    out=wo_sb, in_=wo.rearrange("(k p) m -> p k m", p=128))

                # ---- Q,K feature-major [128, 4096]; V token-major + ones ----
                qT = ap_.tile([128, B * S], BF16, name="qT")
                kT = ap_.tile([128, B * S], BF16, name="kT")
                for dst, wsb, bsb in ((qT, wq_sb, bq_sb), (kT, wk_sb, bk_sb)):
                    for b in range(B):
                        for tt in range(4):
                            c0 = b * S + tt * 512
                            acc = psA.tile([128, 512], F32, name="acc")
                            for kc in range(8):
                                nc.tensor.matmul(
                                    acc, lhsT=wsb[:, kc, b, :],
                                    rhs=xg_sb[:, kc, c0:c0 + 512],
                                    start=(kc == 0), stop=(kc == 7))
                            nc.vector.tensor_scalar_add(
                                out=dst[:, c0:c0 + 512], in0=acc,
                                scalar1=bsb[:, b:b + 1])

                # V token-major; cols 64:128 are ones so the AV matmul also
                # emits the softmax denominator replicated on rows 64:128
                vtok = ap_.tile([128, 32, 2, 128], BF16, name="vtok")
                nc.vector.memset(vtok[:, :, :, 64:128], 1.0)
                for b in range(B):
                    for tt in range(16):
                        acc = psA.tile([128, 128], F32, name="acc")
                        for kc in range(8):
                            nc.tensor.matmul(
                                acc,
                                lhsT=xg_sb[:, kc,
                                           b * S + tt * 128:b * S + tt * 128 + 128],
                                rhs=wv_sb[:, kc, b, :],
                                start=(kc == 0), stop=(kc == 7))
                        nc.vector.tensor_copy(
                            out=vtok[:, b * 16 + tt, :, 0:64],
                            in_=acc.rearrange("p (h c) -> p h c", h=2))

                # ---- causal attention: 2 heads x 2 batches, no max-sub.
                # q blocks of 512, k blocks of 128; the two heads' score
                # matmuls are row-packed (K=64 at PE rows 0/64) and run
                # concurrently in the PE array.
                attnT = ap_.tile([128, B, S], BF16, name="attnT")
                for b in range(B):
                    for q4 in range(4):
                        c0 = b * S + q4 * 512
                        O = [psO.tile([128, 512], F32, name=f"O{hh}")
                             for hh in range(2)]
                        nkt = 4 * q4 + 4

                        def emit_sc(kt, b=b, c0=c0):
                            k0 = b * S + kt * 128
                            sc = psS.tile([128, 2, 512], F32, name="sc")
                            for hh in range(2):
                                hp = hh * 64
                                nc.tensor.matmul(
                                    sc[:, hh, :],
                                    lhsT=kT[hp:hp + 64, k0:k0 + 128],
                                    rhs=qT[hp:hp + 64, c0:c0 + 512],
                                    start=True, stop=True)
                            return sc

                        # software pipeline: scores for kt+1 are queued on
                        # the PE before the exp-dependent AV of kt, so the
                        # PE never idles behind the ACT engine
                        sc_cur = emit_sc(0)
                        for kt in range(nkt):
                            pt_ = ptp.tile([128, 2, 512], BF16, name="pt_")
                            nc.scalar.activation(out=pt_, in_=sc_cur,
                                                 func=AF.Exp)
                            if kt + 1 < nkt:
                                sc_cur = emit_sc(kt + 1)
                            m = kt - 4 * q4
                            if m >= 0:
                                for hh in range(2):
                                    nc.vector.tensor_mul(
                                        out=pt_[:, hh, :], in0=pt_[:, hh, :],
                                        in1=mask_sb[:, m, :])
                            for hh in range(2):
                                nc.tensor.matmul(
                                    O[hh], lhsT=vtok[:, b * 16 + kt, hh, :],
                                    rhs=pt_[:, hh, :],
                                    start=(kt == 0), stop=(kt == nkt - 1))
                        for hh in range(2):
                            rec = rp.tile([64, 512], F32, name="rec")
                            nc.vector.reciprocal(out=rec, in_=O[hh][64:128, :])
                            nc.vector.tensor_mul(
                                out=attnT[hh * 64:hh * 64 + 64, b,
                                          q4 * 512:(q4 + 1) * 512],
                                in0=O[hh][0:64, :], in1=rec)

                # shard j of the A2A = my heads for batch j//4, tokens (j%4)
                nc.sync.dma_start(
                    out=a2a_in.rearrange("(s p t) -> p s t", s=8, p=128),
                    in_=attnT.rearrange("p b (jj t) -> p (b jj) t", jj=4))

            nc.gpsimd.collective_compute(
                "AllToAll", mybir.AluOpType.bypass, replica_groups=rg,
                ins=[a2a_in.opt()], outs=[a2a_out.opt()])

            with (
                tc.tile_pool(name="postp", bufs=1) as pc,
                tc.tile_pool(name="w1p", bufs=4) as w1p,
                tc.tile_pool(name="w2p", bufs=4) as w2p,
            ):
                af_sb = pc.tile([128, 8, TOK], BF16, name="af_sb")
                _af3 = a2a_out.rearrange("(i p t) -> i p t", i=8, p=128)
                for i in range(8):
                    [nc.gpsimd, nc.scalar][i % 2].dma_start(
                        out=af_sb[:, i, :], in_=_af3[i])

                # ---- wo projection + residual into x_tok (in place) ----
                yT = pc.tile([128, 8, TOK], BF16, name="yT")
                for fb in range(8):
                    acc = psA.tile([128, 512], F32, name="acc")
                    for kc in range(8):
                        nc.tensor.matmul(
                            acc, lhsT=wo_sb[:, kc, fb * 128:(fb + 1) * 128],
                            rhs=af_sb[:, kc, :],
                            start=(kc == 0), stop=(kc == 7))
                    nc.vector.tensor_scalar_add(out=yT[:, fb, :], in0=acc,
                                                scalar1=bo_sb[:, fb:fb + 1])
                for fb in range(8):
                    for t in range(4):
                        pt = psA.tile([128, 128], BF16, name="acc")
                        nc.tensor.transpose(pt, yT[:, fb, t * 128:(t + 1) * 128],
                                            id_sb)
                        nc.vector.tensor_add(
                            out=x_tok[:, t, fb * 128:(fb + 1) * 128],
                            in0=x_tok[:, t, fb * 128:(fb + 1) * 128], in1=pt)

                # ---- LN2 -> lnT (reused), FFN ----
                layernorm_into_lnT(x_tok)

                h1T = pc.tile([128, 32, TOK], BF16, name="h1T")
                for hbk in range(32):
                    w1t = w1p.tile([128, 8, 128], BF16, name="w1t")
                    nc.sync.dma_start(
                        out=w1t,
                        in_=w1[:, hbk * 128:(hbk + 1) * 128]
                        .rearrange("(k p) m -> p k m", p=128))
                    acc = psA.tile([128, 512], F32, name="acc")
                    for kc in range(8):
                        nc.tensor.matmul(acc, lhsT=w1t[:, kc, :],
                                         rhs=lnT[:, kc, :],
                                         start=(kc == 0), stop=(kc == 7))
                    nc.scalar.activation(out=h1T[:, hbk, :], in_=acc,
                                         func=AF.Gelu,
                                         bias=b1_sb[:, hbk:hbk + 1], scale=1.0)

                for fb in range(8):
                    acc = psA.tile([128, 512], F32, name="acc")
                    for hg in range(4):
                        w2t = w2p.tile([128, 8, 128], BF16, name="w2t")
                        nc.sync.dma_start(
                            out=w2t,
                            in_=w2[hg * 1024:(hg + 1) * 1024,
                                   fb * 128:(fb + 1) * 128]
                            .rearrange("(k p) m -> p k m", p=128))
                        for kc in range(8):
                            nc.tensor.matmul(acc, lhsT=w2t[:, kc, :],
                                             rhs=h1T[:, hg * 8 + kc, :],
                                             start=(hg == 0 and kc == 0),
                                             stop=(hg == 3 and kc == 7))
                    y2T = stg.tile([128, TOK], BF16, name="y2T")
                    nc.vector.tensor_scalar_add(out=y2T, in0=acc,
                                                scalar1=b2_sb[:, fb:fb + 1])
                    for t in range(4):
                        pt = psA.tile([128, 128], BF16, name="acc")
                        nc.tensor.transpose(pt, y2T[:, t * 128:(t + 1) * 128],
                                            id_sb)
                        nc.vector.tensor_add(
                            out=x_tok[:, t, fb * 128:(fb + 1) * 128],
                            in0=x_tok[:, t, fb * 128:(fb + 1) * 128], in1=pt)

                nc.sync.dma_start(out=out.rearrange("(t p) d -> p t d", p=128),
                                  in_=x_tok)
    nc.compile()
    return nc


_NC_CACHE = {}


def _get_nc():
    if "nc" not in _NC_CACHE:
        _NC_CACHE["nc"] = build_nc()
    return _NC_CACHE["nc"]


def _prep_in_maps(x, ln1_g, ln1_b, wq, bq, wk, bk, wv, bv, wo, bo,
                  ln2_g, ln2_b, w1, b1, w2, b2):
    bf16 = ml_dtypes.bfloat16
    f32 = np.float32
    x = np.asarray(x, f32)
    DK = 64
    sc = 1.0 / np.sqrt(DK)
    ln1_g = np.asarray(ln1_g, f32)
    ln1_b = np.asarray(ln1_b, f32)
    ln2_g = np.asarray(ln2_g, f32)
    ln2_b = np.asarray(ln2_b, f32)
    wq = np.asarray(wq, f32)
    wk = np.asarray(wk, f32)
    wv = np.asarray(wv, f32)
    wo_np = np.asarray(wo, f32)
    w1 = np.asarray(w1, f32)
    w2 = np.asarray(w2, f32)

    wq_f = (ln1_g[:, None] * wq * sc).astype(bf16)
    bq_f = ((ln1_b @ wq + np.asarray(bq, f32)) * sc).astype(f32)
    wk_f = (ln1_g[:, None] * wk).astype(bf16)
    bk_f = (ln1_b @ wk + np.asarray(bk, f32)).astype(f32)
    wv_f = (ln1_g[:, None] * wv).astype(bf16)
    bv_f = (ln1_b @ wv + np.asarray(bv, f32)).astype(f32)
    bo_f = (np.asarray(bo, f32) + bv_f @ wo_np).astype(f32)
    wo_f = wo_np.astype(bf16)
    w1_f = (ln2_g[:, None] * w1).astype(bf16)
    b1_f = (ln2_b @ w1 + np.asarray(b1, f32)).astype(f32)
    w2_f = w2.astype(bf16)
    b2_f = np.asarray(b2, f32)

    # masks[i, m, j] = 1 iff q position j (within a 512 block) >= k position
    # i + 128*m (k block m of the diagonal 512-token region)
    ii = np.arange(128)[:, None, None]
    mm_ = np.arange(4)[None, :, None]
    jj = np.arange(512)[None, None, :]
    masks_np = (jj >= ii + 128 * mm_).astype(f32).astype(bf16)
    id128 = np.eye(128, dtype=f32).astype(bf16)

    in_maps = []
    for core in range(NCORES):
        g, l = divmod(core, GRP)

        def hsel(b, j=core):
            m = (j + 4 * b) % 8
            return slice(m * 128, (m + 1) * 128)

        wo_perm = np.concatenate(
            [wo_f[((i + 4 * g) % 8) * 128:((i + 4 * g) % 8) * 128 + 128, :]
             for i in range(8)], axis=0)
        in_maps.append({
            "x_own": np.ascontiguousarray(x[g, l * TOK:(l + 1) * TOK, :]),
            "wq": np.ascontiguousarray(
                np.stack([wq_f[:, hsel(b)] for b in range(B)], axis=1)),
            "wk": np.ascontiguousarray(
                np.stack([wk_f[:, hsel(b)] for b in range(B)], axis=1)),
            "wv": np.ascontiguousarray(
                np.stack([wv_f[:, hsel(b)] for b in range(B)], axis=1)),
            "bq": np.ascontiguousarray(
                np.stack([bq_f[hsel(b)] for b in range(B)])),
            "bk": np.ascontiguousarray(
                np.stack([bk_f[hsel(b)] for b in range(B)])),
            "wo": np.ascontiguousarray(wo_perm), "bo": bo_f,
            "w1": w1_f, "b1": b1_f, "w2": w2_f, "b2": b2_f,
            "id128": id128, "masks": masks_np,
        })
    return in_maps


def kernel(**inputs):
    nc = _get_nc()
    in_maps = _prep_in_maps(**inputs)
    res = run_bass_kernel_spmd(nc, in_maps, core_ids=list(range(NCORES)))
    full = np.empty((B, S, D), np.float32)
    for core in range(NCORES):
        g, l = divmod(core, GRP)
        full[g, l * TOK:(l + 1) * TOK, :] = res.results[core]["out"]
    return full



# revision 13
# speedup vs baseline: 1.2160x; 1.0805x over previous
"""Trainium2 Bass kernel: dense transformer block (LN1-attn-LN2-FFN, causal, 16 heads).

Sharding (8 NeuronCores, SPMD one graph):
  - core j: token-parallel for LN/FFN/residual: owns tokens [512l, 512(l+1))
    of batch g, where g, l = divmod(j, 4)
  - attention head-parallel with cyclic head-batch assignment: core j computes
    head pair {2m, 2m+1}, m = (j + 4b) % 8, for EACH batch b over the full
    2048-token sequence. Uniform causal loop structure on every core; all
    per-core variation (which heads / which tokens) lives in the input data.
  - comm: 8-core AllGather of LN1^T output (QKV sees all tokens), 8-core
    AllToAll of normalized attention^T (head-shard -> token-shard). The
    receive-side head permutation is folded into host-permuted wo rows.
  - matmuls bf16 (f32 accumulate); residual stream f32; softmax without
    max-subtraction (scores are O(1) for this problem scale).
  - LN gains/biases, 1/sqrt(dk), and bv are folded into weights host-side.
"""

import numpy as np
import ml_dtypes

import concourse.bass as bass
import concourse.tile as tile
from concourse import bacc, mybir
from concourse.bass_utils import run_bass_kernel_spmd

F32 = mybir.dt.float32
BF16 = mybir.dt.bfloat16
AF = mybir.ActivationFunctionType

D = 1024
DFF = 4096
B = 2
S = 2048
NCORES = 8
GRP = 4
TOK = 512        # tokens per core (FFN/LN shard)
EPS = 1e-5

AG_IN = D * TOK          # bf16 elems contributed to xln AllGather
A2A_N = NCORES * 128 * TOK   # total elems in the 8-way AllToAll


def build_nc():
    nc = bacc.Bacc("TRN2", target_bir_lowering=False, debug=False,
                   num_devices=NCORES)

    x_own = nc.dram_tensor("x_own", [TOK, D], F32, kind="ExternalInput").ap()
    wq = nc.dram_tensor("wq", [D, B, 128], BF16, kind="ExternalInput").ap()
    wk = nc.dram_tensor("wk", [D, B, 128], BF16, kind="ExternalInput").ap()
    wv = nc.dram_tensor("wv", [D, B, 128], BF16, kind="ExternalInput").ap()
    bq = nc.dram_tensor("bq", [B, 128], F32, kind="ExternalInput").ap()
    bk = nc.dram_tensor("bk", [B, 128], F32, kind="ExternalInput").ap()
    wo = nc.dram_tensor("wo", [D, D], BF16, kind="ExternalInput").ap()
    bo = nc.dram_tensor("bo", [D], F32, kind="ExternalInput").ap()
    w1 = nc.dram_tensor("w1", [D, DFF], BF16, kind="ExternalInput").ap()
    b1 = nc.dram_tensor("b1", [DFF], F32, kind="ExternalInput").ap()
    w2 = nc.dram_tensor("w2", [DFF, D], BF16, kind="ExternalInput").ap()
    b2 = nc.dram_tensor("b2", [D], F32, kind="ExternalInput").ap()
    id128 = nc.dram_tensor("id128", [128, 128], BF16, kind="ExternalInput").ap()
    masks = nc.dram_tensor("masks", [128, 4, 512], BF16,
                           kind="ExternalInput").ap()
    out = nc.dram_tensor("out", [TOK, D], F32, kind="ExternalOutput").ap()

    rg = [list(range(NCORES))]

    with tile.TileContext(nc) as tc:
        with (
            tc.tile_pool(name="persist", bufs=1) as pp,
            tc.tile_pool(name="stage", bufs=2) as stg,
            tc.tile_pool(name="stats", bufs=4) as stp,
            tc.tile_pool(name="ptp", bufs=6) as ptp,
            tc.tile_pool(name="rp", bufs=2) as rp,
            tc.tile_pool(name="psA", bufs=2, space="PSUM") as psA,
            tc.tile_pool(name="psS", bufs=2, space="PSUM") as psS,
            tc.tile_pool(name="psO", bufs=1, space="PSUM") as psO,
            tc.tile_pool(name="dram", bufs=1, space="DRAM") as dp,
        ):
            # ---- constants ----
            id_sb = pp.tile([128, 128], BF16, name="id_sb")
            nc.sync.dma_start(out=id_sb, in_=id128)
            mask_sb = pp.tile([128, 4, 512], BF16, name="mask_sb")
            nc.sync.dma_start(out=mask_sb, in_=masks)
            eps_sb = pp.tile([128, 1], F32, name="eps_sb")
            nc.vector.memset(eps_sb, EPS)
            bo_sb = pp.tile([128, 8], F32, name="bo_sb")
            nc.sync.dma_start(out=bo_sb, in_=bo.rearrange("(k p) -> p k", p=128))
            b1_sb = pp.tile([128, 32], F32, name="b1_sb")
            nc.sync.dma_start(out=b1_sb, in_=b1.rearrange("(k p) -> p k", p=128))
            b2_sb = pp.tile([128, 8], F32, name="b2_sb")
            nc.sync.dma_start(out=b2_sb, in_=b2.rearrange("(k p) -> p k", p=128))

            x_tok = pp.tile([128, 4, D], F32, name="x_tok")
            nc.sync.dma_start(out=x_tok,
                              in_=x_own.rearrange("(t p) d -> p t d", p=128))

            lnT = pp.tile([128, 8, TOK], BF16, name="lnT")

            def layernorm_into_lnT(src_tile):
                # src_tile: [128, 4, 1024] f32 token-major residual stream
                for t in range(4):
                    xin = src_tile[:, t, :]
                    xg = xin.rearrange("p (g d) -> p g d", g=2)
                    stats = stp.tile([128, 2, 6], F32, name="stats")
                    for gsub in range(2):
                        nc.vector.bn_stats(out=stats[:, gsub, :],
                                           in_=xg[:, gsub, :])
                    mv = stp.tile([128, 2], F32, name="mv")
                    nc.vector.bn_aggr(out=mv, in_=stats)
                    rstd = stp.tile([128, 1], F32, name="rstd")
                    nc.scalar.activation(out=rstd, in_=mv[:, 1:2], func=AF.Sqrt,
                                         bias=eps_sb, scale=1.0)
                    nc.vector.reciprocal(out=rstd, in_=rstd)
                    xln = stg.tile([128, D], BF16, name="xln")
                    nc.vector.tensor_scalar(out=xln, in0=xin, scalar1=mv[:, 0:1],
                                            scalar2=rstd,
                                            op0=mybir.AluOpType.subtract,
                                            op1=mybir.AluOpType.mult)
                    for fb in range(8):
                        pt = psA.tile([128, 128], BF16, name="acc")
                        nc.tensor.transpose(pt, xln[:, fb * 128:(fb + 1) * 128],
                                            id_sb)
                        nc.vector.tensor_copy(
                            out=lnT[:, fb, t * 128:(t + 1) * 128], in_=pt)

            layernorm_into_lnT(x_tok)

            # ---- AllGather xln^T across all 8 cores ----
            ag_in = dp.tile([AG_IN], BF16, name="ag_in")
            ag_out = dp.tile([NCORES * AG_IN], BF16, name="ag_out",
                             addr_space="Shared")
            _agv = ag_in.rearrange("(a p t) -> a p t", a=8, p=128)
            for a in range(8):
                [nc.sync, nc.scalar][a % 2].dma_start(
                    out=_agv[a], in_=lnT[:, a, :])
            nc.gpsimd.collective_compute(
                "AllGather", mybir.AluOpType.bypass, replica_groups=rg,
                ins=[ag_in.opt()], outs=[ag_out.opt()])

            a2a_in = dp.tile([A2A_N], BF16, name="a2a_in")
            a2a_out = dp.tile([A2A_N], BF16, name="a2a_out")

            with tc.tile_pool(name="attnp", bufs=1) as ap_:
                xg_sb = ap_.tile([128, 8, NCORES * TOK], BF16, name="xg_sb")
                ag3 = ag_out.rearrange("(r a p t) -> r a p t", r=NCORES, a=8,
                                       p=128)
                _engs = [nc.gpsimd, nc.scalar]
                for r in range(NCORES):
                    _engs[r % 2].dma_start(
                        out=xg_sb[:, :, r * TOK:(r + 1) * TOK]
                        .rearrange("p a t -> p a t"),
                        in_=ag3[r].rearrange("a p t -> p a t"))

                wq_sb = ap_.tile([128, 8, B, 128], BF16, name="wq_sb")
                nc.sync.dma_start(out=wq_sb,
                                  in_=wq.rearrange("(k p) b m -> p k b m", p=128))
                wk_sb = ap_.tile([128, 8, B, 128], BF16, name="wk_sb")
                nc.sync.dma_start(out=wk_sb,
                                  in_=wk.rearrange("(k p) b m -> p k b m", p=128))
                wv_sb = ap_.tile([128, 8, B, 128], BF16, name="wv_sb")
                nc.sync.dma_start(out=wv_sb,
                                  in_=wv.rearrange("(k p) b m -> p k b m", p=128))
                bq_sb = ap_.tile([128, B], F32, name="bq_sb")
                nc.sync.dma_start(out=bq_sb, in_=bq.rearrange("b p -> p b"))
                bk_sb = ap_.tile([128, B], F32, name="bk_sb")
                nc.sync.dma_start(out=bk_sb, in_=bk.rearrange("b p -> p b"))
                wo_sb = pp.tile([128, 8, 1024], BF16, name="wo_sb")
                nc.sync.dma_start(
                    out=wo_sb, in_=wo.rearrange("(k p) m -> p k m", p=128))

                # ---- Q,K feature-major [128, 4096]; V token-major + ones ----
                qT = ap_.tile([128, B * S], BF16, name="qT")
                kT = ap_.tile([128, B * S], BF16, name="kT")
                for dst, wsb, bsb in ((qT, wq_sb, bq_sb), (kT, wk_sb, bk_sb)):
                    for b in range(B):
                        for tt in range(4):
                            c0 = b * S + tt * 512
                            acc = psA.tile([128, 512], F32, name="acc")
                            for kc in range(8):
                                nc.tensor.matmul(
                                    acc, lhsT=wsb[:, kc, b, :],
                                    rhs=xg_sb[:, kc, c0:c0 + 512],
                                    start=(kc == 0), stop=(kc == 7))
                            nc.vector.tensor_scalar_add(
                                out=dst[:, c0:c0 + 512], in0=acc,
                                scalar1=bsb[:, b:b + 1])

                # V token-major; cols 64:128 are ones so the AV matmul also
                # emits the softmax denominator replicated on rows 64:128
                vtok = ap_.tile([128, 32, 2, 128], BF16, name="vtok")
                nc.vector.memset(vtok[:, :, :, 64:128], 1.0)
                for b in range(B):
                    for tt in range(16):
                        acc = psA.tile([128, 128], F32, name="acc")
                        for kc in range(8):
                            nc.tensor.matmul(
                                acc,
                                lhsT=xg_sb[:, kc,
                                           b * S + tt * 128:b * S + tt * 128 + 128],
                                rhs=wv_sb[:, kc, b, :],
                                start=(kc == 0), stop=(kc == 7))
                        nc.vector.tensor_copy(
                            out=vtok[:, b * 16 + tt, :, 0:64],
                            in_=acc.rearrange("p (h c) -> p h c", h=2))

                # ---- causal attention: 2 heads x 2 batches, no max-sub.
                # q blocks of 512, k blocks of 128; the two heads' score
                # matmuls are row-packed (K=64 at PE rows 0/64) and run
                # concurrently in the PE array.
                attnT = ap_.tile([128, B, S], BF16, name="attnT")
                for b in range(B):
                    for q4 in range(4):
                        c0 = b * S + q4 * 512
                        O = [psO.tile([128, 512], F32, name=f"O{hh}")
                             for hh in range(2)]
                        nkt = 4 * q4 + 4

                        def emit_sc(kt, b=b, c0=c0):
                            k0 = b * S + kt * 128
                            sc = psS.tile([128, 2, 512], F32, name="sc")
                            for hh in range(2):
                                hp = hh * 64
                                nc.tensor.matmul(
                                    sc[:, hh, :],
                                    lhsT=kT[hp:hp + 64, k0:k0 + 128],
                                    rhs=qT[hp:hp + 64, c0:c0 + 512],
                                    start=True, stop=True)
                            return sc

                        # software pipeline: scores for kt+1 are queued on
                        # the PE before the exp-dependent AV of kt, so the
                        # PE never idles behind the ACT engine
                        sc_cur = emit_sc(0)
                        for kt in range(nkt):
                            pt_ = ptp.tile([128, 2, 512], BF16, name="pt_")
                            nc.scalar.activation(out=pt_, in_=sc_cur,
                                                 func=AF.Exp)
                            if kt + 1 < nkt:
                                sc_cur = emit_sc(kt + 1)
                            m = kt - 4 * q4
                            if m >= 0:
                                for hh in range(2):
                                    nc.vector.tensor_mul(
                                        out=pt_[:, hh, :], in0=pt_[:, hh, :],
                                        in1=mask_sb[:, m, :])
                            for hh in range(2):
                                nc.tensor.matmul(
                                    O[hh], lhsT=vtok[:, b * 16 + kt, hh, :],
                                    rhs=pt_[:, hh, :],
                                    start=(kt == 0), stop=(kt == nkt - 1))
                        for hh in range(2):
                            den = rp.tile([64, 512], F32, name="den")
                            nc.vector.tensor_copy(out=den, in_=O[hh][64:128, :])
                            rec = rp.tile([64, 512], F32, name="rec")
                            nc.vector.reciprocal_approx_fast(out=rec, in_=den)
                            nc.vector.tensor_mul(
                                out=attnT[hh * 64:hh * 64 + 64, b,
                                          q4 * 512:(q4 + 1) * 512],
                                in0=O[hh][0:64, :], in1=rec)

                # shard j of the A2A = my heads for batch j//4, tokens (j%4)
                nc.sync.dma_start(
                    out=a2a_in.rearrange("(s p t) -> p s t", s=8, p=128),
                    in_=attnT.rearrange("p b (jj t) -> p (b jj) t", jj=4))

            nc.gpsimd.collective_compute(
                "AllToAll", mybir.AluOpType.bypass, replica_groups=rg,
                ins=[a2a_in.opt()], outs=[a2a_out.opt()])

            with (
                tc.tile_pool(name="postp", bufs=1) as pc,
                tc.tile_pool(name="w1p", bufs=4) as w1p,
                tc.tile_pool(name="w2p", bufs=4) as w2p,
            ):
                af_sb = pc.tile([128, 8, TOK], BF16, name="af_sb")
                _af3 = a2a_out.rearrange("(i p t) -> i p t", i=8, p=128)
                for i in range(8):
                    [nc.gpsimd, nc.scalar][i % 2].dma_start(
                        out=af_sb[:, i, :], in_=_af3[i])

                # ---- wo projection + residual into x_tok (in place) ----
                yT = pc.tile([128, 8, TOK], BF16, name="yT")
                for fb in range(8):
                    acc = psA.tile([128, 512], F32, name="acc")
                    for kc in range(8):
                        nc.tensor.matmul(
                            acc, lhsT=wo_sb[:, kc, fb * 128:(fb + 1) * 128],
                            rhs=af_sb[:, kc, :],
                            start=(kc == 0), stop=(kc == 7))
                    nc.vector.tensor_scalar_add(out=yT[:, fb, :], in0=acc,
                                                scalar1=bo_sb[:, fb:fb + 1])
                for fb in range(8):
                    for t in range(4):
                        pt = psA.tile([128, 128], BF16, name="acc")
                        nc.tensor.transpose(pt, yT[:, fb, t * 128:(t + 1) * 128],
                                            id_sb)
                        nc.vector.tensor_add(
                            out=x_tok[:, t, fb * 128:(fb + 1) * 128],
                            in0=x_tok[:, t, fb * 128:(fb + 1) * 128], in1=pt)

                # ---- LN2 -> lnT (reused), FFN ----
                layernorm_into_lnT(x_tok)

                h1T = pc.tile([128, 32, TOK], BF16, name="h1T")
                for hbk in range(32):
                    w1t = w1p.tile([128, 8, 128], BF16, name="w1t")
                    nc.sync.dma_start(
                        out=w1t,
                        in_=w1[:, hbk * 128:(hbk + 1) * 128]
                        .rearrange("(k p) m -> p k m", p=128))
                    acc = psA.tile([128, 512], F32, name="acc")
                    for kc in range(8):
                        nc.tensor.matmul(acc, lhsT=w1t[:, kc, :],
                                         rhs=lnT[:, kc, :],
                                         start=(kc == 0), stop=(kc == 7))
                    nc.scalar.activation(out=h1T[:, hbk, :], in_=acc,
                                         func=AF.Gelu,
                                         bias=b1_sb[:, hbk:hbk + 1], scale=1.0)

                for fb in range(8):
                    acc = psA.tile([128, 512], F32, name="acc")
                    for hg in range(4):
                        w2t = w2p.tile([128, 8, 128], BF16, name="w2t")
                        nc.sync.dma_start(
                            out=w2t,
                            in_=w2[hg * 1024:(hg + 1) * 1024,
                                   fb * 128:(fb + 1) * 128]
                            .rearrange("(k p) m -> p k m", p=128))
                        for kc in range(8):
                            nc.tensor.matmul(acc, lhsT=w2t[:, kc, :],
                                             rhs=h1T[:, hg * 8 + kc, :],
                                             start=(hg == 0 and kc == 0),
                                             stop=(hg == 3 and kc == 7))
                    y2T = stg.tile([128, TOK], BF16, name="y2T")
                    nc.vector.tensor_scalar_add(out=y2T, in0=acc,
                                                scalar1=b2_sb[:, fb:fb + 1])
                    for t in range(4):
                        pt = psA.tile([128, 128], BF16, name="acc")
                        nc.tensor.transpose(pt, y2T[:, t * 128:(t + 1) * 128],
                                            id_sb)
                        nc.vector.tensor_add(
                            out=x_tok[:, t, fb * 128:(fb + 1) * 128],
                            in0=x_tok[:, t, fb * 128:(fb + 1) * 128], in1=pt)

                nc.sync.dma_start(out=out.rearrange("(t p) d -> p t d", p=128),
                                  in_=x_tok)
    nc.compile()
    return nc


_NC_CACHE = {}


def _get_nc():
    if "nc" not in _NC_CACHE:
        _NC_CACHE["nc"] = build_nc()
    return _NC_CACHE["nc"]


def _prep_in_maps(x, ln1_g, ln1_b, wq, bq, wk, bk, wv, bv, wo, bo,
                  ln2_g, ln2_b, w1, b1, w2, b2):
    bf16 = ml_dtypes.bfloat16
    f32 = np.float32
    x = np.asarray(x, f32)
    DK = 64
    sc = 1.0 / np.sqrt(DK)
    ln1_g = np.asarray(ln1_g, f32)
    ln1_b = np.asarray(ln1_b, f32)
    ln2_g = np.asarray(ln2_g, f32)
    ln2_b = np.asarray(ln2_b, f32)
    wq = np.asarray(wq, f32)
    wk = np.asarray(wk, f32)
    wv = np.asarray(wv, f32)
    wo_np = np.asarray(wo, f32)
    w1 = np.asarray(w1, f32)
    w2 = np.asarray(w2, f32)

    wq_f = (ln1_g[:, None] * wq * sc).astype(bf16)
    bq_f = ((ln1_b @ wq + np.asarray(bq, f32)) * sc).astype(f32)
    wk_f = (ln1_g[:, None] * wk).astype(bf16)
    bk_f = (ln1_b @ wk + np.asarray(bk, f32)).astype(f32)
    wv_f = (ln1_g[:, None] * wv).astype(bf16)
    bv_f = (ln1_b @ wv + np.asarray(bv, f32)).astype(f32)
    bo_f = (np.asarray(bo, f32) + bv_f @ wo_np).astype(f32)
    wo_f = wo_np.astype(bf16)
    w1_f = (ln2_g[:, None] * w1).astype(bf16)
    b1_f = (ln2_b @ w1 + np.asarray(b1, f32)).astype(f32)
    w2_f = w2.astype(bf16)
    b2_f = np.asarray(b2, f32)

    # masks[i, m, j] = 1 iff q position j (within a 512 block) >= k position
    # i + 128*m (k block m of the diagonal 512-token region)
    ii = np.arange(128)[:, None, None]
    mm_ = np.arange(4)[None, :, None]
    jj = np.arange(512)[None, None, :]
    masks_np = (jj >= ii + 128 * mm_).astype(f32).astype(bf16)
    id128 = np.eye(128, dtype=f32).astype(bf16)

    in_maps = []
    for core in range(NCORES):
        g, l = divmod(core, GRP)

        def hsel(b, j=core):
            m = (j + 4 * b) % 8
            return slice(m * 128, (m + 1) * 128)

        wo_perm = np.concatenate(
            [wo_f[((i + 4 * g) % 8) * 128:((i + 4 * g) % 8) * 128 + 128, :]
             for i in range(8)], axis=0)
        in_maps.append({
            "x_own": np.ascontiguousarray(x[g, l * TOK:(l + 1) * TOK, :]),
            "wq": np.ascontiguousarray(
                np.stack([wq_f[:, hsel(b)] for b in range(B)], axis=1)),
            "wk": np.ascontiguousarray(
                np.stack([wk_f[:, hsel(b)] for b in range(B)], axis=1)),
            "wv": np.ascontiguousarray(
                np.stack([wv_f[:, hsel(b)] for b in range(B)], axis=1)),
            "bq": np.ascontiguousarray(
                np.stack([bq_f[hsel(b)] for b in range(B)])),
            "bk": np.ascontiguousarray(
                np.stack([bk_f[hsel(b)] for b in range(B)])),
            "wo": np.ascontiguousarray(wo_perm), "bo": bo_f,
            "w1": w1_f, "b1": b1_f, "w2": w2_f, "b2": b2_f,
            "id128": id128, "masks": masks_np,
        })
    return in_maps


def kernel(**inputs):
    nc = _get_nc()
    in_maps = _prep_in_maps(**inputs)
    res = run_bass_kernel_spmd(nc, in_maps, core_ids=list(range(NCORES)))
    full = np.empty((B, S, D), np.float32)
    for core in range(NCORES):
        g, l = divmod(core, GRP)
        full[g, l * TOK:(l + 1) * TOK, :] = res.results[core]["out"]
    return full

